# revision 4
# baseline (speedup 1.0000x reference)
"""BiLSTM (B=32, S=512, I=H=1024) Trainium2 kernel over 8 NeuronCores.

Tensor-parallel over the gate dimension (each core owns a 128-row H-slice
and its four gate blocks); both directions run as two anti-phased chains
on every core:

  - per-direction PSUM gate banks (walrus requires matmul dst partition 0);
    gate order [i|f|o|g]: one sigmoid over (32,384) + one tanh per dir,
    4 DVE ops per dir (true-tanh formulation).
  - per-step per-direction AllGather of the PE-transposed h-slice.  The
    per-engine instruction order is PINNED with explicit deps so the Tile
    scheduler cannot re-serialize the directions:
      PE:   inj_f, inj_b, mm_f x8, mm_b x8, tp_f, tp_b, xproj, inj(t+1)...
      Sync: xpt(t+1) x2, ccin_f, ccin_b, hT_f, hT_b   (critical DMAs only)
      ACT:  tanh_g_f, sig_ifo_f, tanh_g_b, sig_ifo_b, tanh_c_f, tanh_c_b
      DVE:  u_f, v_f, c_f, u_b, v_b, c_b, h_f, cast_f, h_b, cast_b
    Bulk DMAs (out stores, xproj traffic) ride the idle GpSimd SWDGE queue
    so they never head-of-line block the critical Sync FIFO.
  - xp inject via identity matmul (start=True) pre-runs during the
    exchange window; xpt tiles are prefetched one step ahead.
  - x-projection chunks interleave through the recurrence (one per 4
    steps per direction, consumed from opposite sequence ends) to keep
    the PE HAM-warm and hide their cost in exchange gaps.
  - output written per-step as (S, 64, 128) and un-reversed on the host.
"""

S_FIXED = 512
LAST_EXEC_NS = None

import numpy as np

import concourse.bass as bass
import concourse.bacc as bacc
import concourse.mybir as mybir
import concourse.tile as tile
from concourse.tile_rust import add_dep_helper

# The axon client has no /dev/neuron*, so the driver's NC/routing maps are
# unavailable.  A plausible identity map is fine for client-side validation.
import concourse.libnrt as _libnrt

try:
    _libnrt.get_trn2_nc_mapping()
except Exception:
    _libnrt.get_trn2_nc_mapping = lambda: {(0, i): i for i in range(8)}
try:
    _libnrt.get_device_id_to_routing_id_mapping()
except Exception:
    _fake_rid_map = lambda: {i: i for i in range(16)}
    _libnrt.get_device_id_to_routing_id_mapping = _fake_rid_map
    import concourse.bass_interp as _bi
    import concourse.replica_groups as _rg

    _bi.get_device_id_to_routing_id_mapping = _fake_rid_map
    _rg.get_device_id_to_routing_id_mapping = _fake_rid_map

P = 128
B = 32
B2 = 2 * B
I_DIM = 1024
H_DIM = 1024
NCORES = 8
KCH = H_DIM // P          # 8 k-chunks of the hidden dim
GS = 4 * H_DIM // NCORES  # 512 gate rows per core, order [i|f|o|g]
F32 = mybir.dt.float32
F32R = mybir.dt.float32r
SIG = mybir.ActivationFunctionType.Sigmoid
TANH = mybir.ActivationFunctionType.Tanh


def host_prep(x, W_ii, W_hi, b_i, W_ii_r, W_hi_r, b_i_r, S):
    """Build the 8 per-core input maps."""
    x = np.asarray(x, np.float32)
    # xT[i, s*B+b] = x[b, s, i]
    xT = np.ascontiguousarray(x.transpose(2, 1, 0).reshape(I_DIM, S * B))

    def slices(W, bvec, core):
        # gate rows for this core, in-slice order [i|f|o|g]
        r = np.arange(core * P, core * P + P)
        rows = np.concatenate([r, H_DIM + r, 3 * H_DIM + r, 2 * H_DIM + r])
        Ws = W[rows, :].astype(np.float32)
        bs = bvec[rows].astype(np.float32)
        return np.ascontiguousarray(Ws.T), bs.reshape(1, GS).copy()

    id32 = np.eye(B, dtype=np.float32)
    ones128 = np.ones((1, P), dtype=np.float32)
    in_maps = []
    for c in range(NCORES):
        wiT_f, bias_f = slices(np.asarray(W_ii), np.asarray(b_i), c)
        whT_f, _ = slices(np.asarray(W_hi), np.asarray(b_i), c)
        wiT_b, bias_b = slices(np.asarray(W_ii_r), np.asarray(b_i_r), c)
        whT_b, _ = slices(np.asarray(W_hi_r), np.asarray(b_i_r), c)
        in_maps.append({
            "xT": xT,
            "wiT_f": wiT_f, "whT_f": whT_f, "bias_f": bias_f,
            "wiT_b": wiT_b, "whT_b": whT_b, "bias_b": bias_b,
            "id32": id32, "id32f": id32, "ones128": ones128,
        })
    return in_maps


def host_assemble(results, S):
    """results[c]["out"]: (S, 64, P) -> full (B, S, 2H)."""
    out = np.empty((B, S, 2 * H_DIM), np.float32)
    for c in range(NCORES):
        o = results[c]["out"]  # (S, 64, P); rows 0:32 fwd@t, 32:64 bwd@S-1-t
        out[:, :, c * P:(c + 1) * P] = o[:, 0:B, :].transpose(1, 0, 2)
        out[:, :, H_DIM + c * P:H_DIM + (c + 1) * P] = (
            o[::-1, B:B2, :].transpose(1, 0, 2)
        )
    return out


def build_kernel(S, interleave_xproj=True):
    nc = bacc.Bacc(None)
    SB = S * B
    MCH = SB // P  # sb-chunks of 128 (4 timesteps each)

    xT_e = nc.declare_dram_parameter("xT", [I_DIM, SB], F32R, isOutput=False)
    w_e = {}
    for d in ("f", "b"):
        w_e["wiT_" + d] = nc.declare_dram_parameter("wiT_" + d, [I_DIM, GS], F32R, isOutput=False)
        w_e["whT_" + d] = nc.declare_dram_parameter("whT_" + d, [H_DIM, GS], F32R, isOutput=False)
        w_e["bias_" + d] = nc.declare_dram_parameter("bias_" + d, [1, GS], F32R, isOutput=False)
    id32_e = nc.declare_dram_parameter("id32", [B, B], F32R, isOutput=False)
    id32f_e = nc.declare_dram_parameter("id32f", [B, B], F32, isOutput=False)
    ones_e = nc.declare_dram_parameter("ones128", [1, P], F32R, isOutput=False)
    out_e = nc.declare_dram_parameter("out", [S, B2, P], F32, isOutput=True)

    xp_d = nc.dram_tensor("xp_scratch", [2, S, B, GS], F32R)

    # per-engine emission-order pinning
    last_on = {}

    def pin(engine, inst):
        prev = last_on.get(engine)
        if prev is not None:
            add_dep_helper(inst.ins, prev.ins, sync=False,
                           reason=f"pin {engine} order")
        last_on[engine] = inst
        return inst

    with tile.TileContext(nc) as tc:
        with (
            tc.tile_pool(name="const", bufs=1) as constp,
            tc.tile_pool(name="xsb", bufs=2) as xsbp,
            tc.tile_pool(name="xpt_st", bufs=2) as xpst,
            tc.tile_pool(name="psumB", bufs=2, space="PSUM") as psumB,
            tc.tile_pool(name="psumCf", bufs=2, space="PSUM") as psumCf,
            tc.tile_pool(name="psumCb", bufs=2, space="PSUM") as psumCb,
            tc.tile_pool(name="psumT", bufs=1, space="PSUM") as psumT,
            tc.tile_pool(name="state", bufs=1) as statep,
            tc.tile_pool(name="step", bufs=3) as stepp,
            tc.tile_pool(name="hcomm", bufs=2) as hcommp,
            tc.tile_pool(name="dram", bufs=2, space="DRAM") as dramp,
        ):
            # ---- constants / weights in SBUF ----
            id32 = constp.tile([B, B], F32R, tag="id32", name="id32")
            nc.gpsimd.dma_start(id32[:], id32_e[:])
            id32f = constp.tile([B, B], F32, tag="id32f", name="id32f")
            nc.gpsimd.dma_start(id32f[:], id32f_e[:])
            ones128 = constp.tile([1, P], F32R, tag="ones", name="ones")
            nc.gpsimd.dma_start(ones128[:], ones_e[:])
            wiT = {}
            whT = {}
            biasT = {}
            for d in ("f", "b"):
                wiT[d] = constp.tile([P, KCH, GS], F32R, tag="wiT" + d, name="wiT" + d)
                nc.gpsimd.dma_start(
                    wiT[d][:],
                    w_e["wiT_" + d][:].rearrange("(k p) g -> p k g", p=P),
                )
                whT[d] = constp.tile([P, KCH, GS], F32R, tag="whT" + d, name="whT" + d)
                nc.gpsimd.dma_start(
                    whT[d][:],
                    w_e["whT_" + d][:].rearrange("(k p) g -> p k g", p=P),
                )
                biasT[d] = constp.tile([1, GS], F32R, tag="bias" + d, name="bias" + d)
                nc.gpsimd.dma_start(biasT[d][:], w_e["bias_" + d][:])

            # ---- x-projection chunk: 4 timesteps x one dir -> xp_d ----
            xp_store = {}

            def xproj_chunk(d, m):
                di = 0 if d == "f" else 1
                xsb = xsbp.tile([P, KCH, P], F32R, tag="xsb", name="xsb")
                nc.gpsimd.dma_start(
                    xsb[:],
                    xT_e[:, m * P:(m + 1) * P].rearrange("(k p) c -> p k c", p=P),
                )
                ps = psumB.tile([P, GS], F32, tag="psB", name="psB")
                pin("pe", nc.tensor.matmul(ps[:], ones128[:], biasT[d][:],
                                           start=True, stop=False))
                for k in range(KCH):
                    pin("pe", nc.tensor.matmul(ps[:], xsb[:, k, :],
                                               wiT[d][:, k, :],
                                               start=False, stop=(k == KCH - 1)))
                xpt = xpst.tile([P, GS], F32R, tag="xpt", name="xpt")
                pin("dve", nc.vector.tensor_copy(xpt[:], ps[:]))
                st = nc.gpsimd.dma_start(
                    xp_d[di, 4 * m:4 * m + 4].rearrange("s b g -> (s b) g"),
                    xpt[:],
                )
                xp_store[(di, m)] = st

            # ---- recurrence ----
            c_state = {d: statep.tile([B, P], F32, tag="c" + d, name="c" + d)
                       for d in ("f", "b")}
            for d in ("f", "b"):
                nc.vector.memset(c_state[d][:], 0.0)

            hT_prev = {"f": None, "b": None}
            psum_pool = {"f": psumCf, "b": psumCb}
            xpt_tile = {}

            def load_xpt(t):
                """Prefetch both dirs' xp rows for step t."""
                spos = {"f": t, "b": S - 1 - t}
                for d in ("f", "b"):
                    di = 0 if d == "f" else 1
                    xpt = stepp.tile([B, GS], F32R, tag="xp" + d, name="xp" + d)
                    ld = pin("sync", nc.sync.dma_start(xpt[:], xp_d[di, spos[d]]))
                    if interleave_xproj:
                        add_dep_helper(ld.ins, xp_store[(di, spos[d] // 4)].ins,
                                       sync=True, reason="xp ready")
                    xpt_tile[d] = xpt

            def step(t):
                xpt = dict(xpt_tile)
                load_xpt(t + 1) if t + 1 < S else None
                ps = {}
                acts = {}
                # -- injects then both dirs' recurrent matmul groups --
                for d in ("f", "b"):
                    ps[d] = psum_pool[d].tile([B, GS], F32, tag="ps" + d,
                                              name="ps" + d)
                    pin("pe", nc.tensor.matmul(ps[d][:], id32[:], xpt[d][:],
                                               start=True, stop=(t == 0)))
                for d in ("f", "b"):
                    if t > 0:
                        hT = hT_prev[d]
                        for k in range(KCH):
                            pin("pe", nc.tensor.matmul(ps[d][:], hT[:, k, :],
                                                       whT[d][:, k, :],
                                                       start=False,
                                                       stop=(k == KCH - 1)))
                # -- activations: [i|f|o|g] --
                for d in ("f", "b"):
                    acts[d] = stepp.tile([B, GS], F32, tag="acts" + d,
                                         name="acts" + d)
                    pin("act", nc.scalar.activation(acts[d][:, 3 * P:4 * P],
                                                    ps[d][:, 3 * P:4 * P], TANH))
                    pin("act", nc.scalar.activation(acts[d][:, 0:3 * P],
                                                    ps[d][:, 0:3 * P], SIG))
                u = {}
                v = {}
                for d in ("f", "b"):
                    u[d] = stepp.tile([B, P], F32, tag="u" + d, name="u" + d)
                    pin("dve", nc.vector.tensor_mul(u[d][:], acts[d][:, 0:P],
                                                    acts[d][:, 3 * P:4 * P]))
                    v[d] = stepp.tile([B, P], F32, tag="v" + d, name="v" + d)
                    pin("dve", nc.vector.tensor_mul(v[d][:], acts[d][:, P:2 * P],
                                                    c_state[d][:]))
                    pin("dve", nc.vector.tensor_add(c_state[d][:], u[d][:],
                                                    v[d][:]))
                # -- h, output, transpose, exchange launch (f fully, then b) --
                h = {}
                tp = {}
                hT_self = {}
                cc_out = {}
                for d in ("f", "b"):
                    tc_t = stepp.tile([B, P], F32, tag="tc" + d, name="tc" + d)
                    pin("act", nc.scalar.activation(tc_t[:], c_state[d][:], TANH))
                    h[d] = stepp.tile([B, P], F32, tag="h" + d, name="h" + d)
                    pin("dve", nc.vector.tensor_mul(h[d][:],
                                                    acts[d][:, 2 * P:3 * P],
                                                    tc_t[:]))
                    row0 = 0 if d == "f" else B
                    nc.gpsimd.dma_start(out_e[t, row0:row0 + B], h[d][:])
                    if t == S - 1:
                        continue
                    tp[d] = psumT.tile([P, B], F32, tag="tp" + d, name="tp" + d)
                    pin("pe", nc.tensor.transpose(tp[d][:], h[d][:], id32f[:]))
                    hT_self[d] = hcommp.tile([P, B], F32R, tag="hs" + d,
                                             name="hs" + d)
                    pin("dve", nc.vector.tensor_copy(hT_self[d][:], tp[d][:]))
                    cc_in = dramp.tile([P, B], F32R, tag="ci" + d, name="ci" + d)
                    pin("sync", nc.sync.dma_start(cc_in[:], hT_self[d][:]))
                    cc_out[d] = dramp.tile([NCORES * P, B], F32R, tag="co" + d,
                                           name="co" + d)
                    nc.gpsimd.collective_compute(
                        "AllGather",
                        mybir.AluOpType.bypass,
                        ins=[cc_in[:].opt()],
                        outs=[cc_out[d][:].opt()],
                        replica_groups=[list(range(NCORES))],
                    )
                if t == S - 1:
                    return
                for d in ("f", "b"):
                    hT = hcommp.tile([P, KCH, B], F32R, tag="hr" + d,
                                     name="hr" + d)
                    pin("sync", nc.sync.dma_start(
                        hT[:],
                        cc_out[d][:].rearrange("(k p) b -> p k b", p=P),
                    ))
                    hT_prev[d] = hT

            if interleave_xproj:
                # prologue: the chunks the first steps consume
                for m in (0, 1):
                    xproj_chunk("f", m)
                for m in (MCH - 1, MCH - 2):
                    xproj_chunk("b", m)
                load_xpt(0)
                for t in range(S):
                    step(t)
                    # PE filler AFTER the step's transposes
                    if t % 4 == 0 and t // 4 + 2 < MCH:
                        xproj_chunk("f", t // 4 + 2)
                    elif t % 4 == 2 and MCH - 3 - t // 4 >= 0:
                        xproj_chunk("b", MCH - 3 - t // 4)
            else:
                for m in range(MCH):
                    xproj_chunk("f", m)
                    xproj_chunk("b", m)
                load_xpt(0)
                for t in range(S):
                    step(t)

    return nc


def fix_drain_waits(nc):
    """This walrus build allows only 1 sync-wait per instruction (2 on
    EventSemaphore).  Move excess waits onto EventSemaphore insts placed
    immediately before the instruction on the same engine."""
    ctr = 0
    for fn in nc.m.functions:
        for bb in fn.blocks:
            insts = list(bb.instructions)
            new = []
            changed = False
            for ins in insts:
                si = ins.sync_info
                if (
                    not isinstance(ins, mybir.InstEventSemaphore)
                    and si is not None
                    and len(si.on_wait) > 1
                ):
                    waits = list(si.on_wait)
                    keep, extra = waits[:1], waits[1:]
                    for i in range(0, len(extra), 2):
                        w = mybir.InstEventSemaphore(
                            name=f"I-dwfix-{ctr}",
                            engine=ins.engine,
                            ins=[],
                            outs=[],
                            sync_info=mybir.SyncInfo(
                                on_wait=extra[i : i + 2], on_update=[]
                            ),
                        )
                        ctr += 1
                        new.append(w)
                    ins.sync_info = mybir.SyncInfo(
                        on_wait=keep, on_update=list(si.on_update)
                    )
                    changed = True
                new.append(ins)
            if changed:
                try:
                    bb.instructions = new
                except Exception:
                    bb.instructions.clear()
                    bb.instructions.extend(new)


def kernel(x, W_ii, W_hi, b_i, W_ii_reverse, W_hi_reverse, b_i_reverse):
    """Full inputs in, full (B, S, 2H) output out."""
    import os

    global LAST_EXEC_NS
    import concourse.bass_utils as bu

    bu.upload_artifacts = lambda tmpdir: "local://" + tmpdir
    from concourse.bass_utils import run_bass_kernel_spmd

    S = S_FIXED
    trace = os.environ.get("TRNLSTM_TRACE", "0") == "1"
    interleave = os.environ.get("TRNLSTM_INTERLEAVE", "1") == "1"

    nc = build_kernel(S, interleave_xproj=interleave)
    nc.compile()
    fix_drain_waits(nc)
    in_maps = host_prep(x, W_ii, W_hi, b_i,
                        W_ii_reverse, W_hi_reverse, b_i_reverse, S)
    res = run_bass_kernel_spmd(nc, in_maps, list(range(NCORES)), trace=trace)
    LAST_EXEC_NS = res.exec_time_ns
    return host_assemble(res.results, S)


# revision 5
# speedup vs baseline: 1.1625x; 1.1625x over previous
"""BiLSTM (B=32, S=512, I=H=1024) Trainium2 kernel over 8 NeuronCores.

Tensor-parallel over the gate dimension (each core owns a 128-row H-slice
and its four gate blocks); both directions run as two anti-phased chains
on every core:

  - per-direction PSUM gate banks (walrus requires matmul dst partition 0);
    gate order [i|f|o|g]: one sigmoid over (32,384) + one tanh per dir,
    4 DVE ops per dir (true-tanh formulation).
  - per-step per-direction AllGather of the PE-transposed h-slice.  The
    per-engine instruction order is PINNED with explicit deps so the Tile
    scheduler cannot re-serialize the directions:
      PE:   inj_f, inj_b, mm_f x8, mm_b x8, tp_f, tp_b, xproj, inj(t+1)...
      Sync: xpt(t+1) x2, ccin_f, ccin_b, hT_f, hT_b   (critical DMAs only)
      ACT:  tanh_g_f, sig_ifo_f, tanh_g_b, sig_ifo_b, tanh_c_f, tanh_c_b
      DVE:  u_f, v_f, c_f, u_b, v_b, c_b, h_f, cast_f, h_b, cast_b
    Bulk DMAs (out stores, xproj traffic) ride the idle GpSimd SWDGE queue
    so they never head-of-line block the critical Sync FIFO.
  - xp inject via identity matmul (start=True) pre-runs during the
    exchange window; xpt tiles are prefetched one step ahead.
  - x-projection chunks interleave through the recurrence (one per 4
    steps per direction, consumed from opposite sequence ends) to keep
    the PE HAM-warm and hide their cost in exchange gaps.
  - output written per-step as (S, 64, 128) and un-reversed on the host.
"""

S_FIXED = 512
LAST_EXEC_NS = None

import numpy as np

import concourse.bass as bass
import concourse.bacc as bacc
import concourse.mybir as mybir
import concourse.tile as tile
from concourse.tile_rust import add_dep_helper

# The axon client has no /dev/neuron*, so the driver's NC/routing maps are
# unavailable.  A plausible identity map is fine for client-side validation.
import concourse.libnrt as _libnrt

try:
    _libnrt.get_trn2_nc_mapping()
except Exception:
    _libnrt.get_trn2_nc_mapping = lambda: {(0, i): i for i in range(8)}
try:
    _libnrt.get_device_id_to_routing_id_mapping()
except Exception:
    _fake_rid_map = lambda: {i: i for i in range(16)}
    _libnrt.get_device_id_to_routing_id_mapping = _fake_rid_map
    import concourse.bass_interp as _bi
    import concourse.replica_groups as _rg

    _bi.get_device_id_to_routing_id_mapping = _fake_rid_map
    _rg.get_device_id_to_routing_id_mapping = _fake_rid_map

P = 128
B = 32
B2 = 2 * B
I_DIM = 1024
H_DIM = 1024
NCORES = 8
KCH = H_DIM // P          # 8 k-chunks of the hidden dim
GS = 4 * H_DIM // NCORES  # 512 gate rows per core, order [i|f|o|g]
F32 = mybir.dt.float32
F32R = mybir.dt.float32r
SIG = mybir.ActivationFunctionType.Sigmoid
TANH = mybir.ActivationFunctionType.Tanh


def host_prep(x, W_ii, W_hi, b_i, W_ii_r, W_hi_r, b_i_r, S):
    """Build the 8 per-core input maps."""
    x = np.asarray(x, np.float32)
    # xT[i, s*B+b] = x[b, s, i]
    xT = np.ascontiguousarray(x.transpose(2, 1, 0).reshape(I_DIM, S * B))

    def slices(W, bvec, core):
        # gate rows for this core, in-slice order [i|f|o|g]
        r = np.arange(core * P, core * P + P)
        rows = np.concatenate([r, H_DIM + r, 3 * H_DIM + r, 2 * H_DIM + r])
        Ws = W[rows, :].astype(np.float32)
        bs = bvec[rows].astype(np.float32)
        return np.ascontiguousarray(Ws.T), bs.reshape(1, GS).copy()

    id32 = np.eye(B, dtype=np.float32)
    ones128 = np.ones((1, P), dtype=np.float32)
    in_maps = []
    for c in range(NCORES):
        wiT_f, bias_f = slices(np.asarray(W_ii), np.asarray(b_i), c)
        whT_f, _ = slices(np.asarray(W_hi), np.asarray(b_i), c)
        wiT_b, bias_b = slices(np.asarray(W_ii_r), np.asarray(b_i_r), c)
        whT_b, _ = slices(np.asarray(W_hi_r), np.asarray(b_i_r), c)
        in_maps.append({
            "xT": xT,
            "wiT_f": wiT_f, "whT_f": whT_f, "bias_f": bias_f,
            "wiT_b": wiT_b, "whT_b": whT_b, "bias_b": bias_b,
            "id32": id32, "id32f": id32, "ones128": ones128,
        })
    return in_maps


def host_assemble(results, S):
    """results[c]["out"]: (S, 64, P) -> full (B, S, 2H)."""
    out = np.empty((B, S, 2 * H_DIM), np.float32)
    for c in range(NCORES):
        o = results[c]["out"]  # (S, 64, P); rows 0:32 fwd@t, 32:64 bwd@S-1-t
        out[:, :, c * P:(c + 1) * P] = o[:, 0:B, :].transpose(1, 0, 2)
        out[:, :, H_DIM + c * P:H_DIM + (c + 1) * P] = (
            o[::-1, B:B2, :].transpose(1, 0, 2)
        )
    return out


def build_kernel(S, interleave_xproj=True):
    nc = bacc.Bacc(None)
    SB = S * B
    MCH = SB // P  # sb-chunks of 128 (4 timesteps each)

    xT_e = nc.declare_dram_parameter("xT", [I_DIM, SB], F32R, isOutput=False)
    w_e = {}
    for d in ("f", "b"):
        w_e["wiT_" + d] = nc.declare_dram_parameter("wiT_" + d, [I_DIM, GS], F32R, isOutput=False)
        w_e["whT_" + d] = nc.declare_dram_parameter("whT_" + d, [H_DIM, GS], F32R, isOutput=False)
        w_e["bias_" + d] = nc.declare_dram_parameter("bias_" + d, [1, GS], F32R, isOutput=False)
    id32_e = nc.declare_dram_parameter("id32", [B, B], F32R, isOutput=False)
    id32f_e = nc.declare_dram_parameter("id32f", [B, B], F32, isOutput=False)
    ones_e = nc.declare_dram_parameter("ones128", [1, P], F32R, isOutput=False)
    out_e = nc.declare_dram_parameter("out", [S, B2, P], F32, isOutput=True)

    xp_d = nc.dram_tensor("xp_scratch", [2, S, B, GS], F32R)

    # per-engine emission-order pinning
    last_on = {}

    def pin(engine, inst):
        prev = last_on.get(engine)
        if prev is not None:
            add_dep_helper(inst.ins, prev.ins, sync=False,
                           reason=f"pin {engine} order")
        last_on[engine] = inst
        return inst

    with tile.TileContext(nc) as tc:
        with (
            tc.tile_pool(name="const", bufs=1) as constp,
            tc.tile_pool(name="xsb", bufs=2) as xsbp,
            tc.tile_pool(name="xpt_st", bufs=2) as xpst,
            tc.tile_pool(name="psumB", bufs=2, space="PSUM") as psumB,
            tc.tile_pool(name="psumCf", bufs=2, space="PSUM") as psumCf,
            tc.tile_pool(name="psumCb", bufs=2, space="PSUM") as psumCb,
            tc.tile_pool(name="psumT", bufs=1, space="PSUM") as psumT,
            tc.tile_pool(name="state", bufs=1) as statep,
            tc.tile_pool(name="step", bufs=3) as stepp,
            tc.tile_pool(name="hcomm", bufs=2) as hcommp,
            tc.tile_pool(name="dram", bufs=2, space="DRAM") as dramp,
        ):
            # ---- constants / weights in SBUF ----
            id32 = constp.tile([B, B], F32R, tag="id32", name="id32")
            nc.gpsimd.dma_start(id32[:], id32_e[:])
            id32f = constp.tile([B, B], F32, tag="id32f", name="id32f")
            nc.gpsimd.dma_start(id32f[:], id32f_e[:])
            ones128 = constp.tile([1, P], F32R, tag="ones", name="ones")
            nc.gpsimd.dma_start(ones128[:], ones_e[:])
            wiT = {}
            whT = {}
            biasT = {}
            for d in ("f", "b"):
                wiT[d] = constp.tile([P, KCH, GS], F32R, tag="wiT" + d, name="wiT" + d)
                nc.gpsimd.dma_start(
                    wiT[d][:],
                    w_e["wiT_" + d][:].rearrange("(k p) g -> p k g", p=P),
                )
                whT[d] = constp.tile([P, KCH, GS], F32R, tag="whT" + d, name="whT" + d)
                nc.gpsimd.dma_start(
                    whT[d][:],
                    w_e["whT_" + d][:].rearrange("(k p) g -> p k g", p=P),
                )
                biasT[d] = constp.tile([1, GS], F32R, tag="bias" + d, name="bias" + d)
                nc.gpsimd.dma_start(biasT[d][:], w_e["bias_" + d][:])

            # ---- x-projection chunk: 4 timesteps x one dir -> xp_d ----
            xp_store = {}

            def xproj_chunk(d, m):
                di = 0 if d == "f" else 1
                xsb = xsbp.tile([P, KCH, P], F32R, tag="xsb", name="xsb")
                nc.gpsimd.dma_start(
                    xsb[:],
                    xT_e[:, m * P:(m + 1) * P].rearrange("(k p) c -> p k c", p=P),
                )
                ps = psumB.tile([P, GS], F32, tag="psB", name="psB")
                pin("pe", nc.tensor.matmul(ps[:], ones128[:], biasT[d][:],
                                           start=True, stop=False))
                for k in range(KCH):
                    pin("pe", nc.tensor.matmul(ps[:], xsb[:, k, :],
                                               wiT[d][:, k, :],
                                               start=False, stop=(k == KCH - 1)))
                xpt = xpst.tile([P, GS], F32R, tag="xpt", name="xpt")
                pin("dve", nc.vector.tensor_copy(xpt[:], ps[:]))
                st = nc.gpsimd.dma_start(
                    xp_d[di, 4 * m:4 * m + 4].rearrange("s b g -> (s b) g"),
                    xpt[:],
                )
                xp_store[(di, m)] = st

            # ---- recurrence ----
            c_state = {d: statep.tile([B, P], F32, tag="c" + d, name="c" + d)
                       for d in ("f", "b")}
            for d in ("f", "b"):
                nc.vector.memset(c_state[d][:], 0.0)

            hT_prev = {"f": None, "b": None}
            psum_pool = {"f": psumCf, "b": psumCb}
            xpt_tile = {}

            def load_xpt(t):
                """Prefetch both dirs' xp rows for step t."""
                spos = {"f": t, "b": S - 1 - t}
                for d in ("f", "b"):
                    di = 0 if d == "f" else 1
                    xpt = stepp.tile([B, GS], F32R, tag="xp" + d, name="xp" + d)
                    ld = pin("sync", nc.sync.dma_start(xpt[:], xp_d[di, spos[d]]))
                    if interleave_xproj:
                        add_dep_helper(ld.ins, xp_store[(di, spos[d] // 4)].ins,
                                       sync=True, reason="xp ready")
                    xpt_tile[d] = xpt

            def step(t):
                xpt = dict(xpt_tile)
                load_xpt(t + 1) if t + 1 < S else None
                ps = {}
                acts = {}
                # -- injects then both dirs' recurrent matmul groups --
                for d in ("f", "b"):
                    ps[d] = psum_pool[d].tile([B, GS], F32, tag="ps" + d,
                                              name="ps" + d)
                    pin("pe", nc.tensor.matmul(ps[d][:], id32[:], xpt[d][:],
                                               start=True, stop=(t == 0)))
                for d in ("f", "b"):
                    if t > 0:
                        hT = hT_prev[d]
                        col0 = 0 if d == "f" else B
                        for k in range(KCH):
                            pin("pe", nc.tensor.matmul(
                                ps[d][:], hT[:, k, col0:col0 + B],
                                whT[d][:, k, :],
                                start=False, stop=(k == KCH - 1)))
                # -- activations: [i|f|o|g] --
                for d in ("f", "b"):
                    acts[d] = stepp.tile([B, GS], F32, tag="acts" + d,
                                         name="acts" + d)
                    pin("act", nc.scalar.activation(acts[d][:, 3 * P:4 * P],
                                                    ps[d][:, 3 * P:4 * P], TANH))
                    pin("act", nc.scalar.activation(acts[d][:, 0:3 * P],
                                                    ps[d][:, 0:3 * P], SIG))
                u = {}
                v = {}
                for d in ("f", "b"):
                    u[d] = stepp.tile([B, P], F32, tag="u" + d, name="u" + d)
                    pin("dve", nc.vector.tensor_mul(u[d][:], acts[d][:, 0:P],
                                                    acts[d][:, 3 * P:4 * P]))
                    v[d] = stepp.tile([B, P], F32, tag="v" + d, name="v" + d)
                    pin("dve", nc.vector.tensor_mul(v[d][:], acts[d][:, P:2 * P],
                                                    c_state[d][:]))
                    pin("dve", nc.vector.tensor_add(c_state[d][:], u[d][:],
                                                    v[d][:]))
                # -- h, output, one merged transpose+AllGather per step --
                h = {}
                hT_self = None
                if t < S - 1:
                    hT_self = hcommp.tile([P, B2], F32R, tag="hs", name="hs")
                for d in ("f", "b"):
                    tc_t = stepp.tile([B, P], F32, tag="tc" + d, name="tc" + d)
                    pin("act", nc.scalar.activation(tc_t[:], c_state[d][:], TANH))
                    h[d] = stepp.tile([B, P], F32, tag="h" + d, name="h" + d)
                    pin("dve", nc.vector.tensor_mul(h[d][:],
                                                    acts[d][:, 2 * P:3 * P],
                                                    tc_t[:]))
                    row0 = 0 if d == "f" else B
                    nc.gpsimd.dma_start(out_e[t, row0:row0 + B], h[d][:])
                    if t == S - 1:
                        continue
                    tp = psumT.tile([P, B], F32, tag="tp" + d, name="tp" + d)
                    pin("pe", nc.tensor.transpose(tp[:], h[d][:], id32f[:]))
                    col0 = 0 if d == "f" else B
                    pin("dve", nc.vector.tensor_copy(
                        hT_self[:, col0:col0 + B], tp[:]))
                if t == S - 1:
                    return
                cc_in = dramp.tile([P, B2], F32R, tag="ci", name="ci")
                pin("sync", nc.sync.dma_start(cc_in[:], hT_self[:]))
                cc_out = dramp.tile([NCORES * P, B2], F32R, tag="co", name="co")
                nc.gpsimd.collective_compute(
                    "AllGather",
                    mybir.AluOpType.bypass,
                    ins=[cc_in[:].opt()],
                    outs=[cc_out[:].opt()],
                    replica_groups=[list(range(NCORES))],
                )
                hT = hcommp.tile([P, KCH, B2], F32R, tag="hr", name="hr")
                pin("sync", nc.sync.dma_start(
                    hT[:],
                    cc_out[:].rearrange("(k p) b -> p k b", p=P),
                ))
                hT_prev["f"] = hT
                hT_prev["b"] = hT

            if interleave_xproj:
                # prologue: the chunks the first steps consume
                for m in (0, 1):
                    xproj_chunk("f", m)
                for m in (MCH - 1, MCH - 2):
                    xproj_chunk("b", m)
                load_xpt(0)
                for t in range(S):
                    step(t)
                    # PE filler AFTER the step's transposes
                    if t % 4 == 0 and t // 4 + 2 < MCH:
                        xproj_chunk("f", t // 4 + 2)
                    elif t % 4 == 2 and MCH - 3 - t // 4 >= 0:
                        xproj_chunk("b", MCH - 3 - t // 4)
            else:
                for m in range(MCH):
                    xproj_chunk("f", m)
                    xproj_chunk("b", m)
                load_xpt(0)
                for t in range(S):
                    step(t)

    return nc


def fix_drain_waits(nc):
    """This walrus build allows only 1 sync-wait per instruction (2 on
    EventSemaphore).  Move excess waits onto EventSemaphore insts placed
    immediately before the instruction on the same engine."""
    ctr = 0
    for fn in nc.m.functions:
        for bb in fn.blocks:
            insts = list(bb.instructions)
            new = []
            changed = False
            for ins in insts:
                si = ins.sync_info
                if (
                    not isinstance(ins, mybir.InstEventSemaphore)
                    and si is not None
                    and len(si.on_wait) > 1
                ):
                    waits = list(si.on_wait)
                    keep, extra = waits[:1], waits[1:]
                    for i in range(0, len(extra), 2):
                        w = mybir.InstEventSemaphore(
                            name=f"I-dwfix-{ctr}",
                            engine=ins.engine,
                            ins=[],
                            outs=[],
                            sync_info=mybir.SyncInfo(
                                on_wait=extra[i : i + 2], on_update=[]
                            ),
                        )
                        ctr += 1
                        new.append(w)
                    ins.sync_info = mybir.SyncInfo(
                        on_wait=keep, on_update=list(si.on_update)
                    )
                    changed = True
                new.append(ins)
            if changed:
                try:
                    bb.instructions = new
                except Exception:
                    bb.instructions.clear()
                    bb.instructions.extend(new)


def kernel(x, W_ii, W_hi, b_i, W_ii_reverse, W_hi_reverse, b_i_reverse):
    """Full inputs in, full (B, S, 2H) output out."""
    import os

    global LAST_EXEC_NS
    import concourse.bass_utils as bu

    bu.upload_artifacts = lambda tmpdir: "local://" + tmpdir
    from concourse.bass_utils import run_bass_kernel_spmd

    S = S_FIXED
    trace = os.environ.get("TRNLSTM_TRACE", "0") == "1"
    interleave = os.environ.get("TRNLSTM_INTERLEAVE", "1") == "1"

    nc = build_kernel(S, interleave_xproj=interleave)
    nc.compile()
    fix_drain_waits(nc)
    in_maps = host_prep(x, W_ii, W_hi, b_i,
                        W_ii_reverse, W_hi_reverse, b_i_reverse, S)
    res = run_bass_kernel_spmd(nc, in_maps, list(range(NCORES)), trace=trace)
    LAST_EXEC_NS = res.exec_time_ns
    return host_assemble(res.results, S)


# revision 6
# speedup vs baseline: 1.3888x; 1.1947x over previous
"""BiLSTM (B=32, S=512, I=H=1024) Trainium2 kernel over 8 NeuronCores.

Strategy: tensor-parallel over the gate dimension (each core owns a 128-row
H-slice and its four gate blocks), both directions interleaved on all 8
cores.  Per step, each core computes its gate slice as float32r matmuls
accumulated in PSUM (xp injected via an identity matmul, tanh folded into
sigmoid by host-side 2x weight scaling), updates its c/h slice, PE-transposes
the h-slice and exchanges it with the other cores via AllGather collectives.
x is projected on-device from a host-pretransposed xT; outputs are
reassembled on the host.
"""

KERNEL_COMM = "collective"  # "rdma" | "collective4" | "collective2" | "collective"
S_FIXED = 512

LAST_EXEC_NS = None

"""BiLSTM TRN2 kernel: TP=8 over the gate dimension, per-step h all-gather.

Layout decisions (host prepares everything):
  - Each core c owns H-slice c (rows c*128..c*128+127 of the hidden dim).
    Its gate rows, in-slice order [i | f | o | g]: i: c*128+[0:128) of block 0,
    f: block 1, o: block 3, g: block 2 (g's W/b rows pre-scaled x2 so that
    tanh(x) = 2*sigmoid(2x)-1 needs only the one big sigmoid).
  - xT (I, S*B) s-major is fed to every core; x_proj computed on device into
    DRAM xp[d, s, b, gs], bias folded in via a rank-1 matmul.
  - Recurrent step: gates_psum(32, GS) = id32 @ xp_t + sum_k hT[k] @ whT[k],
    all matmuls in float32r.  One sigmoid over (32, GS).  DVE state update.
    h(32,128) -> PE-transpose -> hT_self(128,32) -> exchange -> hT_recv(128,8,32).
  - fwd and bwd interleaved step-by-step on all 8 cores.
"""

import numpy as np

import concourse.bass as bass
import concourse.bacc as bacc
import concourse.mybir as mybir
import concourse.tile as tile
from concourse import library_config
from concourse.tile_rust import add_dep_helper

# The axon client has no /dev/neuron*, so the driver's NC/routing maps are
# unavailable.  Relative-dest remote DMA descriptors don't bake these values
# into the NEFF, so a plausible identity map is fine for client-side
# validation and the simulator.
import concourse.libnrt as _libnrt

try:
    _libnrt.get_trn2_nc_mapping()
except Exception:
    _libnrt.get_trn2_nc_mapping = lambda: {(0, i): i for i in range(8)}
try:
    _libnrt.get_device_id_to_routing_id_mapping()
except Exception:
    _fake_rid_map = lambda: {i: i for i in range(16)}
    _libnrt.get_device_id_to_routing_id_mapping = _fake_rid_map
    import concourse.bass_interp as _bi
    import concourse.replica_groups as _rg

    _bi.get_device_id_to_routing_id_mapping = _fake_rid_map
    _rg.get_device_id_to_routing_id_mapping = _fake_rid_map

P = 128
B = 32
I_DIM = 1024
H_DIM = 1024
NCORES = 8
KCH = H_DIM // P          # 8 k-chunks of the hidden dim
GS = 4 * H_DIM // NCORES  # 512 gate rows per core
F32 = mybir.dt.float32
F32R = mybir.dt.float32r
SIG = mybir.ActivationFunctionType.Sigmoid


def r(ap):
    """View an fp32 AP as float32r for the PE."""
    return ap.bitcast(F32R)


def host_prep(x, W_ii, W_hi, b_i, W_ii_r, W_hi_r, b_i_r, S):
    """Build the 8 per-core input maps."""
    x = np.asarray(x, np.float32)
    # xT[i, s*B+b] = x[b, s, i]
    xT = np.ascontiguousarray(x.transpose(2, 1, 0).reshape(I_DIM, S * B))

    def slices(W, bvec, core):
        # gate rows for this core, order [i|f|o|g] (true tanh on g)
        rows_i = np.arange(core * P, core * P + P)
        rows_f = H_DIM + rows_i
        rows_g = 2 * H_DIM + rows_i
        rows_o = 3 * H_DIM + rows_i
        rows = np.concatenate([rows_i, rows_f, rows_o, rows_g])
        Ws = W[rows, :].astype(np.float32).copy()
        bs = bvec[rows].astype(np.float32).copy()
        # transpose -> (K, GS)
        return np.ascontiguousarray(Ws.T), bs.reshape(1, GS)

    id32 = np.eye(B, dtype=np.float32)
    ones128 = np.ones((1, P), dtype=np.float32)
    in_maps = []
    for c in range(NCORES):
        wiT, bias = slices(np.asarray(W_ii), np.asarray(b_i), c)
        whT, _ = slices(np.asarray(W_hi), np.asarray(b_i), c)
        wiT_r, bias_r = slices(np.asarray(W_ii_r), np.asarray(b_i_r), c)
        whT_r, _ = slices(np.asarray(W_hi_r), np.asarray(b_i_r), c)
        in_maps.append({
            "xT": xT,
            "wiT_f": wiT, "whT_f": whT, "bias_f": bias,
            "wiT_b": wiT_r, "whT_b": whT_r, "bias_b": bias_r,
            "id32": id32, "id32f": id32, "ones128": ones128,
        })
    return in_maps


def host_assemble(results, S):
    """results[c]["out"]: (2, S, B, P) -> full (B, S, 2H)."""
    out = np.empty((B, S, 2 * H_DIM), np.float32)
    for c in range(NCORES):
        o = results[c]["out"]  # (2, S, B, P)
        out[:, :, c * P:(c + 1) * P] = o[0].transpose(1, 0, 2)
        out[:, :, H_DIM + c * P:H_DIM + (c + 1) * P] = o[1].transpose(1, 0, 2)
    return out


def build_kernel(S, comm="collective", xproj_jit_chunk=None, rel_wait=True):
    """Emit the SPMD kernel; returns nc.

    comm: "collective" | "rdma"
    """
    nc = bacc.Bacc(None)
    SB = S * B
    MCH = SB // P  # sb-chunks of 128

    xT_e = nc.declare_dram_parameter("xT", [I_DIM, SB], F32R, isOutput=False)
    w_e = {}
    for d in ("f", "b"):
        w_e["wiT_" + d] = nc.declare_dram_parameter("wiT_" + d, [I_DIM, GS], F32R, isOutput=False)
        w_e["whT_" + d] = nc.declare_dram_parameter("whT_" + d, [H_DIM, GS], F32R, isOutput=False)
        w_e["bias_" + d] = nc.declare_dram_parameter("bias_" + d, [1, GS], F32R, isOutput=False)
    id32_e = nc.declare_dram_parameter("id32", [B, B], F32R, isOutput=False)
    ones_e = nc.declare_dram_parameter("ones128", [1, P], F32R, isOutput=False)
    id32f_e = nc.declare_dram_parameter("id32f", [B, B], F32, isOutput=False)
    out_e = nc.declare_dram_parameter("out", [2, S, B, P], F32, isOutput=True)

    xp_d = nc.dram_tensor("xp_scratch", [2, S, B, GS], F32R)

    with tile.TileContext(nc) as tc:
        with (
            tc.tile_pool(name="const", bufs=1) as constp,
            tc.tile_pool(name="xsb", bufs=3) as xsbp,
            tc.tile_pool(name="xpt_st", bufs=3) as xpst,
            tc.tile_pool(name="psumB", bufs=2, space="PSUM") as psumB,
            tc.tile_pool(name="psumC", bufs=1, space="PSUM") as psumC,
            tc.tile_pool(name="psumT", bufs=1, space="PSUM") as psumT,
            tc.tile_pool(name="state", bufs=1) as statep,
            tc.tile_pool(name="step", bufs=3) as stepp,
            tc.tile_pool(name="hcomm", bufs=2) as hcommp,
            tc.tile_pool(name="dram", bufs=2, space="DRAM") as dramp,
        ):
            if comm == "rdma":
                nc.gpsimd.load_library(library_config.remote_dma)
            # ---- constants / weights in SBUF ----
            id32 = constp.tile([B, B], F32R, tag="id32", name="id32")
            id32f = constp.tile([B, B], F32, tag="id32f", name="id32f")
            nc.sync.dma_start(id32f[:], id32f_e[:])
            nc.sync.dma_start(id32[:], id32_e[:])
            ones128 = constp.tile([1, P], F32R, tag="ones", name="ones")
            nc.sync.dma_start(ones128[:], ones_e[:])
            wiT = {}
            whT = {}
            biasT = {}
            for d in ("f", "b"):
                wiT[d] = constp.tile([P, KCH, GS], F32R, tag="wiT" + d, name="wiT" + d)
                nc.sync.dma_start(
                    wiT[d][:],
                    w_e["wiT_" + d][:].rearrange("(k p) g -> p k g", p=P),
                )
                whT[d] = constp.tile([P, KCH, GS], F32R, tag="whT" + d, name="whT" + d)
                nc.sync.dma_start(
                    whT[d][:],
                    w_e["whT_" + d][:].rearrange("(k p) g -> p k g", p=P),
                )
                biasT[d] = constp.tile([1, GS], F32R, tag="bias" + d, name="bias" + d)
                nc.sync.dma_start(biasT[d][:], w_e["bias_" + d][:])

            # ---- x_proj chunks (interleaved into the recurrence) ----
            xp_store = {}

            def xproj_chunk(m, d):
                xsb = xsbp.tile([P, KCH, P], F32R, tag="xsb", name="xsb")
                nc.sync.dma_start(
                    xsb[:],
                    xT_e[:, m * P:(m + 1) * P].rearrange("(k p) c -> p k c", p=P),
                )
                ps = psumB.tile([P, GS], F32, tag="psB", name="psB")
                nc.tensor.matmul(ps[:], ones128[:], biasT[d][:],
                                 start=True, stop=False)
                for k in range(KCH):
                    nc.tensor.matmul(ps[:], xsb[:, k, :], wiT[d][:, k, :],
                                     start=False, stop=(k == KCH - 1))
                xpt = xpst.tile([P, GS], F32R, tag="xpt", name="xpt")
                nc.vector.tensor_copy(xpt[:], ps[:])
                di = 0 if d == "f" else 1
                s0 = (m * 4)
                st = nc.sync.dma_start(
                    xp_d[di, s0:s0 + 4].rearrange("s b g -> (s b) g"),
                    xpt[:],
                )
                xp_store[(di, m)] = st

            for m in (0, 1):
                xproj_chunk(m, "f")
            for m in (MCH - 1, MCH - 2):
                xproj_chunk(m, "b")

            # ---- phase C: recurrence ----
            c_state = {d: statep.tile([B, P], F32, tag="c_" + d, name="c_" + d) for d in ("f", "b")}
            for d in ("f", "b"):
                nc.vector.memset(c_state[d][:], 0.0)

            if comm == "rdma":
                recv_sem = {d: nc.alloc_semaphore(f"recv_sem_{d}") for d in ("f", "b")}
                rel_sem = {d: nc.alloc_semaphore(f"rel_sem_{d}") for d in ("f", "b")}
                pid_rv = nc.gpsimd.partition_id()
                hT_recv_bufs = {
                    d: [statep.tile([P, KCH, B], F32R, tag=f"hTr_{d}{p}",
                                    name=f"hTr_{d}{p}") for p in (0, 1)]
                    for d in ("f", "b")
                }
                hT_self_bufs = {
                    d: [statep.tile([P, B], F32R, tag=f"hTs_{d}{p}",
                                    name=f"hTs_{d}{p}") for p in (0, 1)]
                    for d in ("f", "b")
                }
                prev_transpose = {"f": None, "b": None}
                last_trigger = {"f": [None, None], "b": [None, None]}
                RDESTS = [None] + [(0, j) for j in range(1, NCORES)]
                pid_dve = nc.vector.partition_id()

            # per-dir rolling hT receive buffer (Tile pools handle reuse deps)
            def step(d, t):
                di = 0 if d == "f" else 1
                spos = t if d == "f" else S - 1 - t
                xpt = stepp.tile([B, GS], F32R, tag="xp_t" + d, name="xp_t" + d)
                ld = nc.sync.dma_start(
                    xpt[:], xp_d[di, spos].rearrange("b g -> b g"))
                add_dep_helper(ld.ins, xp_store[(di, spos // 4)].ins,
                               sync=True, reason="xp ready")
                ps = psumC.tile([B, GS], F32, tag="psC" + d, name="psC" + d)
                nc.tensor.matmul(ps[:], id32[:], xpt[:],
                                 start=True, stop=(t == 0))
                if t > 0:
                    if comm == "rdma":
                        hT = hT_recv_bufs[d][(t - 1) % 2]
                        wait_i = nc.tensor.wait_ge(recv_sem[d], 14 * t)
                        if prev_transpose[d] is not None:
                            add_dep_helper(
                                wait_i.ins, prev_transpose[d].ins, sync=False,
                                reason="rdma: recv-wait after own transpose")
                    else:
                        hT = hT_prev[d]
                    for k in range(KCH):
                        mm = nc.tensor.matmul(ps[:], hT[:, k, :], whT[d][:, k, :],
                                              start=False, stop=(k == KCH - 1))
                        if comm == "rdma" and k == 0:
                            add_dep_helper(mm.ins, wait_i.ins, sync=False,
                                           reason="rdma: matmul after recv-wait")
                acts = stepp.tile([B, GS], F32, tag="acts" + d, name="acts" + d)
                TANH = mybir.ActivationFunctionType.Tanh
                nc.scalar.activation(acts[:, 3 * P:4 * P], ps[:, 3 * P:4 * P],
                                     TANH)
                nc.scalar.activation(acts[:, 0:3 * P], ps[:, 0:3 * P], SIG)
                i_ap = acts[:, 0 * P:1 * P]
                f_ap = acts[:, 1 * P:2 * P]
                o_ap = acts[:, 2 * P:3 * P]
                g_ap = acts[:, 3 * P:4 * P]
                u = stepp.tile([B, P], F32, tag="u" + d, name="u" + d)
                nc.vector.tensor_mul(u[:], i_ap, g_ap)
                v = stepp.tile([B, P], F32, tag="v" + d, name="v" + d)
                nc.vector.tensor_mul(v[:], f_ap, c_state[d][:])
                nc.vector.tensor_add(c_state[d][:], u[:], v[:])
                tc_t = stepp.tile([B, P], F32, tag="tc" + d, name="tc" + d)
                nc.scalar.activation(tc_t[:], c_state[d][:], TANH)
                h = stepp.tile([B, P], F32, tag="h" + d, name="h" + d)
                nc.vector.tensor_mul(h[:], o_ap, tc_t[:])
                nc.sync.dma_start(out_e[di, spos], h[:])
                if t == S - 1:
                    return None
                # transpose h -> (P, B)
                tp = psumT.tile([P, B], F32, tag="tp" + d, name="tp" + d)
                tp_inst = nc.tensor.transpose(tp[:], h[:], id32f[:])
                if comm == "rdma":
                    prev_transpose[d] = tp_inst
                    hT_self = hT_self_bufs[d][t % 2]
                    if rel_wait and t >= 2:
                        # release handshake: round t-2's descriptors drained
                        wr = nc.vector.wait_ge(rel_sem[d], 16 * (t - 1))
                    cp = nc.vector.tensor_copy(hT_self[:], tp[:])
                    if rel_wait and t >= 2:
                        add_dep_helper(cp.ins, wr.ins, sync=False,
                                       reason="rdma: copy after release wait")
                    if last_trigger[d][t % 2] is not None:
                        # WAR vs the SDMA read two steps ago; real safety comes
                        # from the recv-sem transitivity, this just orders the
                        # Tile schedule / race model.
                        add_dep_helper(cp.ins, last_trigger[d][t % 2].ins,
                                       sync=True,
                                       reason="rdma: reuse hT_self after trigger")
                    nc.vector.tensor_copy(
                        hT_recv_bufs[d][t % 2][:, bass.ds(pid_dve, 1), :],
                        hT_self[:])
                    nc.gpsimd.remote_dma_broadcast(
                        out_ap=hT_recv_bufs[d][t % 2][:, bass.ds(pid_rv, 1), :],
                        in_ap=hT_self[:],
                        remote_sem=recv_sem[d],
                        local_sem=rel_sem[d],
                        rdests=RDESTS,
                    )
                    trig = nc.gpsimd.trigger_dma(count=None)
                    last_trigger[d][t % 2] = trig
                    # Scheduler-sim-only stand-ins for the remote/SWDGE
                    # increments; stripped from the final BIR.
                    nc.gpsimd.sem_inc(recv_sem[d], 14)
                    nc.gpsimd.sem_inc(rel_sem[d], 16)
                    return "rdma"
                hT_self = hcommp.tile([P, B], F32R, tag="hself" + d, name="hself" + d)
                nc.vector.tensor_copy(hT_self[:], tp[:])
                # exchange
                if comm == "collective":
                    cc_in = dramp.tile([P, B], F32R, tag="ccin" + d, name="ccin" + d)
                    nc.sync.dma_start(cc_in[:], hT_self[:])
                    cc_out = dramp.tile([NCORES * P, B], F32R, tag="ccout" + d, name="ccout" + d)
                    nc.gpsimd.collective_compute(
                        "AllGather",
                        mybir.AluOpType.bypass,
                        ins=[cc_in[:].opt()],
                        outs=[cc_out[:].opt()],
                        replica_groups=[list(range(NCORES))],
                    )
                    hT = hcommp.tile([P, KCH, B], F32R, tag="hrecv" + d, name="hrecv" + d)
                    nc.sync.dma_start(
                        hT[:],
                        cc_out[:].rearrange("(k p) b -> p k b", p=P),
                    )
                    return hT
                elif comm == "collective4":
                    # two half-slice AllGathers per dir -> 4 in flight per
                    # step-pair; ncfw pipelines concurrent collectives.
                    hT = hcommp.tile([P, KCH, B], F32R, tag="hrecv4" + d,
                                     name="hrecv4" + d)
                    for half in (0, 1):
                        cc_in = dramp.tile([P // 2, B], F32R,
                                           tag=f"ccin4{d}{half}",
                                           name=f"ccin4{d}{half}")
                        nc.sync.dma_start(
                            cc_in[:], hT_self[half * 64:(half + 1) * 64, :])
                        cc_out = dramp.tile([NCORES * P // 2, B], F32R,
                                            tag=f"ccout4{d}{half}",
                                            name=f"ccout4{d}{half}")
                        nc.gpsimd.collective_compute(
                            "AllGather",
                            mybir.AluOpType.bypass,
                            ins=[cc_in[:].opt()],
                            outs=[cc_out[:].opt()],
                            replica_groups=[list(range(NCORES))],
                        )
                        nc.sync.dma_start(
                            hT[half * 64:(half + 1) * 64, :, :],
                            cc_out[:].rearrange("(k p) b -> p k b", p=P // 2),
                        )
                    return hT
                elif comm == "collective2":
                    return hT_self
                else:
                    raise NotImplementedError(comm)

            def exchange_merged(t, hT_self_f, hT_self_b):
                """One AllGather for both directions' hT slices."""
                cc_in = dramp.tile([2 * P, B], F32R, tag="ccin2", name="ccin2")
                nc.sync.dma_start(cc_in[0:P, :], hT_self_f[:])
                nc.sync.dma_start(cc_in[P:2 * P, :], hT_self_b[:])
                cc_out = dramp.tile([NCORES * 2 * P, B], F32R, tag="ccout2",
                                    name="ccout2")
                nc.gpsimd.collective_compute(
                    "AllGather",
                    mybir.AluOpType.bypass,
                    ins=[cc_in[:].opt()],
                    outs=[cc_out[:].opt()],
                    replica_groups=[list(range(NCORES))],
                )
                # cc_out rows: [fwd_0(128); bwd_0(128); fwd_1; bwd_1; ...]
                cc_v = cc_out[:].rearrange("(c d p) b -> d p c b", d=2, p=P)
                hT_f = hcommp.tile([P, KCH, B], F32R, tag="hrecvf", name="hrecvf")
                nc.sync.dma_start(hT_f[:], cc_v[0])
                hT_b = hcommp.tile([P, KCH, B], F32R, tag="hrecvb", name="hrecvb")
                nc.sync.dma_start(hT_b[:], cc_v[1])
                return hT_f, hT_b

            hT_prev = {}
            if comm == "collective2":
                for t in range(S):
                    selfs = {}
                    for d in ("f", "b"):
                        selfs[d] = step(d, t)
                    if selfs["f"] is not None:
                        hT_prev["f"], hT_prev["b"] = exchange_merged(
                            t, selfs["f"], selfs["b"])
            else:
                for t in range(S):
                    for d in ("f", "b"):
                        nxt = step(d, t)
                        if nxt is not None:
                            hT_prev[d] = nxt
                    if t % 4 == 0 and t // 4 + 2 < MCH:
                        xproj_chunk(t // 4 + 2, "f")
                    elif t % 4 == 2 and MCH - 3 - t // 4 >= 0:
                        xproj_chunk(MCH - 3 - t // 4, "b")

    if comm == "rdma":
        _strip_fake_incs(nc, ("recv_sem_f", "recv_sem_b", "rel_sem_f", "rel_sem_b"))
    return nc


def _strip_fake_incs(nc, sem_names):
    """Remove every on_update entry for the given sems (scheduler-sim-only
    stand-ins for remote increments) and drop update-only EventSemaphore
    carriers that become empty."""
    names = set(sem_names)
    for fn in nc.m.functions:
        for bb in fn.blocks:
            new = []
            changed = False
            for ins in bb.instructions:
                si = ins.sync_info
                if si is not None and any(
                    u.ant_name in names for u in si.on_update
                ):
                    kept = [u for u in si.on_update if u.ant_name not in names]
                    ins.sync_info = mybir.SyncInfo(
                        on_wait=list(si.on_wait), on_update=kept
                    )
                    changed = True
                    if (
                        isinstance(ins, mybir.InstEventSemaphore)
                        and not kept
                        and not si.on_wait
                    ):
                        continue  # drop the empty carrier
                new.append(ins)
            if changed:
                try:
                    bb.instructions = new
                except Exception:
                    bb.instructions.clear()
                    bb.instructions.extend(new)


def fix_drain_waits(nc):
    """This walrus build allows only 1 sync-wait per instruction (2 on
    EventSemaphore).  Move excess waits onto EventSemaphore insts placed
    immediately before the instruction on the same engine."""
    ctr = 0
    for fn in nc.m.functions:
        for bb in fn.blocks:
            insts = list(bb.instructions)
            new = []
            changed = False
            for ins in insts:
                si = ins.sync_info
                if (
                    not isinstance(ins, mybir.InstEventSemaphore)
                    and si is not None
                    and len(si.on_wait) > 1
                ):
                    waits = list(si.on_wait)
                    keep, extra = waits[:1], waits[1:]
                    for i in range(0, len(extra), 2):
                        w = mybir.InstEventSemaphore(
                            name=f"I-dwfix-{ctr}",
                            engine=ins.engine,
                            ins=[],
                            outs=[],
                            sync_info=mybir.SyncInfo(
                                on_wait=extra[i : i + 2], on_update=[]
                            ),
                        )
                        ctr += 1
                        new.append(w)
                    ins.sync_info = mybir.SyncInfo(
                        on_wait=keep, on_update=list(si.on_update)
                    )
                    changed = True
                new.append(ins)
            if changed:
                try:
                    bb.instructions = new
                except Exception:
                    bb.instructions.clear()
                    bb.instructions.extend(new)


def kernel(x, W_ii, W_hi, b_i, W_ii_reverse, W_hi_reverse, b_i_reverse):
    """Full inputs in, full (B, S, 2H) output out."""
    import os

    global LAST_EXEC_NS
    import concourse.bass_utils as bu

    bu.upload_artifacts = lambda tmpdir: "local://" + tmpdir
    from concourse.bass_utils import run_bass_kernel_spmd

    S = S_FIXED
    comm = os.environ.get("TRNLSTM_COMM", KERNEL_COMM)
    trace = os.environ.get("TRNLSTM_TRACE", "0") == "1"

    nc = build_kernel(S, comm=comm, rel_wait=False)
    nc.compile()
    fix_drain_waits(nc)
    in_maps = host_prep(x, W_ii, W_hi, b_i,
                        W_ii_reverse, W_hi_reverse, b_i_reverse, S)
    res = run_bass_kernel_spmd(nc, in_maps, list(range(NCORES)), trace=trace)
    LAST_EXEC_NS = res.exec_time_ns
    return host_assemble(res.results, S)

